# revision 1
# baseline (speedup 1.0000x reference)
"""Trainium2 Bass kernel for nn_DLRLoss (top-k masking loss).

Reference computation (per row of input [B, C]):
    top3 values z1 >= z2 >= z3 of the row
    ind  = 1.0 if argmax(row) == target else 0.0
    x_y  = row[target]
    loss = -(x_y - z2*ind - z1*(1-ind)) / (z1 - z3 + EPS)
    return mean(loss)

Strategy: data-parallel over 8 NeuronCores (8192 rows each). Per core,
stream 64 tiles of [128, 1000] through SBUF with a single vector.max
(top-8) pass per tile for z1/z2/z3 — the only full-data compute pass.
x_y is fetched by 4 dma_gather calls (descriptor DMA, 256B aligned
chunks containing each row's target element; int16 gather indices limit
each call to 2048 rows) and extracted with 3 vectorized DVE ops.
The loss algebra runs vectorized on [128, 64]; the kernel returns
per-partition partial sums, host sums 8*128 partials and divides by B.
"""

import numpy as np

B, C = 65536, 1000
N_CORES = 8
BL = B // N_CORES          # rows per core: 8192
P = 128                    # SBUF partitions
NT = BL // P               # tiles per core: 64
EPS = 1e-12

CHUNK = 64                 # f32 elems per gathered chunk (256B, HW minimum)
GROWS = 2048               # rows per dma_gather (int16 chunk ids < 32768)
NG = BL // GROWS           # dma_gather calls per core: 4
GTILES = GROWS // P        # output tiles per gather: 16

_CACHE = {}


def _build():
    import concourse.bass as bass
    import concourse.mybir as mybir
    from concourse.tile import TileContext

    f32 = mybir.dt.float32
    i16 = mybir.dt.int16
    i32 = mybir.dt.int32
    Alu = mybir.AluOpType

    nc = bass.Bass()
    x_in = nc.declare_dram_parameter("x", [BL, C], f32, isOutput=False)
    idx_in = nc.declare_dram_parameter("idx", [P, NG * (GROWS // 16)], i16,
                                       isOutput=False)
    off_in = nc.declare_dram_parameter("off", [P, NT], f32, isOutput=False)
    out_p = nc.declare_dram_parameter("out", [P, 1], f32, isOutput=True)

    # Row-tiles per DMA instruction. HW A/B (paired-slope, 40 trials) showed
    # AGG=4 is within noise of AGG=1 (+5us +/- 17us) — both HBM-limited —
    # and the cost model mildly prefers AGG=1, so keep single-tile loads.
    AGG = 1
    x_tiles = x_in[:, :].rearrange("(m s p) c -> m p s c", p=P, s=AGG)
    x_flat = x_in[:, :].rearrange("a b -> (a b)")

    with TileContext(nc) as tc:
        with (
            tc.tile_pool(name="const", bufs=1) as cpool,
            tc.tile_pool(name="xp", bufs=8) as xpool,
        ):
            # In-chunk index ramp, directly in f32 (0..63 are exact; skips
            # a 4us DVE convert in the kernel tail). InstIota needs the
            # default 'standard' gpsimd library, so emit it before switching
            # to 'mlp' for the dma_gathers.
            iota_f = cpool.tile([P, NT, CHUNK], f32)
            nc.gpsimd.iota(iota_f[:, :, :], pattern=[[0, NT], [1, CHUNK]],
                           base=0, channel_multiplier=0,
                           allow_small_or_imprecise_dtypes=True)
            from concourse import library_config
            nc.gpsimd.load_library(library_config.mlp)

            # --- x_y gather: 4 descriptor-DMA gathers of 256B chunks ---
            idx_sb = cpool.tile([P, NG * (GROWS // 16)], i16)
            nc.sync.dma_start(out=idx_sb[:, :], in_=idx_in[:, :])
            off_sb = cpool.tile([P, NT], f32)
            nc.sync.dma_start(out=off_sb[:, :], in_=off_in[:, :])

            # 512 idxs per instruction: the SWDGE descriptor ring (16KB,
            # 16B/desc) overflows at 2048 idxs; 512 leaves 4x margin.
            GSUB = 512
            SPG = GROWS // GSUB               # sub-gathers per region: 4
            chunks = cpool.tile([P, NT, CHUNK], f32)
            for g in range(NG):
                src = x_flat[g * GROWS * C:(g + 1) * GROWS * C].rearrange(
                    "(n k) -> n k", k=CHUNK)
                for s in range(SPG):
                    q = g * SPG + s
                    nc.gpsimd.dma_gather(
                        out_ap=chunks[:, q * (GSUB // P):(q + 1) * (GSUB // P), :],
                        in_ap=src,
                        idxs_ap=idx_sb[:, g * (GROWS // 16) + s * (GSUB // 16):
                                       g * (GROWS // 16) + (s + 1) * (GSUB // 16)],
                        num_idxs=GSUB,
                        num_idxs_reg=GSUB,
                        elem_size=CHUNK,
                    )

            # Restore the default library so the kernel is re-executable
            # (the next run's iota needs 'standard' loaded).
            nc.gpsimd.load_library(library_config.standard)

            # --- top-8 per row: the single full-data compute pass ---
            top8 = cpool.tile([P, 8 * NT], f32)
            for m in range(NT // AGG):
                xt = xpool.tile([P, AGG, C], f32, tag="x")
                nc.sync.dma_start(out=xt[:, :, :], in_=x_tiles[m])
                for s in range(AGG):
                    j = m * AGG + s
                    nc.vector.max(out=top8[:, 8 * j:8 * (j + 1)],
                                  in_=xt[:, s, :])

            # --- x_y extraction from chunks: 3 vectorized DVE ops ---
            # (Hoisting this before/inside the max8 loop was tried and is
            # SLOWER in the cost model — it head-of-line-blocks DVE on the
            # gather completion. Keep it after the loop.)
            mask = cpool.tile([P, NT, CHUNK], f32)
            off_b = off_sb[:, :].unsqueeze(2).to_broadcast([P, NT, CHUNK])
            nc.vector.tensor_tensor(out=mask[:, :, :], in0=iota_f[:, :, :],
                                    in1=off_b, op=Alu.is_equal)
            nc.vector.tensor_tensor(out=mask[:, :, :], in0=mask[:, :, :],
                                    in1=chunks[:, :, :], op=Alu.mult)
            xyb = cpool.tile([P, NT], f32)
            nc.vector.tensor_reduce(xyb[:, :], mask[:, :, :],
                                    mybir.AxisListType.X, op=Alu.add)

            # --- loss algebra on [P, NT] ---
            t8 = top8[:, :].rearrange("p (j k) -> p j k", k=8)
            z1 = t8[:, :, 0]
            z2 = t8[:, :, 1]
            z3 = t8[:, :, 2]

            ind = cpool.tile([P, NT], f32)
            d21 = cpool.tile([P, NT], f32)
            num = cpool.tile([P, NT], f32)
            den = cpool.tile([P, NT], f32)
            rec = cpool.tile([P, NT], f32)
            q = cpool.tile([P, NT], f32)
            lsum = cpool.tile([P, 1], f32)

            # ind = (x_y >= z1)  (equality iff target is the row argmax)
            nc.vector.tensor_tensor(out=ind[:, :], in0=xyb[:, :], in1=z1,
                                    op=Alu.is_ge)
            # num = (z1 - x_y) + ind * (z2 - z1)
            nc.vector.tensor_tensor(out=d21[:, :], in0=z2, in1=z1,
                                    op=Alu.subtract)
            nc.vector.tensor_tensor(out=num[:, :], in0=z1, in1=xyb[:, :],
                                    op=Alu.subtract)
            nc.vector.tensor_tensor(out=d21[:, :], in0=ind[:, :], in1=d21[:, :],
                                    op=Alu.mult)
            nc.vector.tensor_tensor(out=num[:, :], in0=num[:, :], in1=d21[:, :],
                                    op=Alu.add)
            # den = z1 - z3 + EPS
            nc.vector.tensor_tensor(out=den[:, :], in0=z1, in1=z3,
                                    op=Alu.subtract)
            nc.vector.tensor_scalar_add(den[:, :], den[:, :], EPS)
            # q = num / den via reciprocal + one Newton step:
            # rec' = rec * (2 - den*rec)
            two_t = cpool.tile([P, NT], f32)
            nc.vector.memset(two_t[:, :], 2.0)
            nc.vector.reciprocal(out=rec[:, :], in_=den[:, :])
            nc.vector.tensor_tensor(out=q[:, :], in0=den[:, :], in1=rec[:, :],
                                    op=Alu.mult)
            nc.vector.tensor_tensor(out=q[:, :], in0=two_t[:, :], in1=q[:, :],
                                    op=Alu.subtract)
            nc.vector.tensor_tensor(out=rec[:, :], in0=rec[:, :], in1=q[:, :],
                                    op=Alu.mult)
            nc.vector.tensor_tensor(out=q[:, :], in0=num[:, :], in1=rec[:, :],
                                    op=Alu.mult)
            nc.vector.reduce_sum(lsum[:, :], q[:, :], mybir.AxisListType.X)
            nc.sync.dma_start(out=out_p[:, :], in_=lsum[:, :])

    _legalize_waits(nc, mybir)
    # Populate .instr bytes for extended-inst InstISA subclasses (the
    # manual library reload); raw Bass skips this Bacc pass and the NEFF
    # compiler rejects empty .instr with "ISA wrong length".
    mybir.codegen_inst_isa_subclasses(nc)
    return nc


def _legalize_waits(nc, mybir):
    """walrus's TPB descriptor encodings accept a single sync-wait per
    instruction; Tile sometimes emits 2+. Move surplus waits onto standalone
    event-semaphore instructions executed by the same engine's sequencer
    immediately before (same semantics: sequencer blocks, then dispatches)."""
    for f in nc.m.functions:
        for b in f.blocks:
            il = b.instructions
            new = []
            changed = False
            for i in il:
                si = i.sync_info
                waits = list(si.on_wait) if (si and si.on_wait) else []
                if len(waits) > 1 and type(i).__name__ != "InstEventSemaphore":
                    for k, w in enumerate(waits[:-1]):
                        new.append(mybir.InstEventSemaphore(
                            name=f"{i.name}-evw{k}",
                            engine=i.engine,
                            ins=[], outs=[],
                            bass_nofuse=True,
                            sync_info=mybir.SyncInfo(on_wait=[w],
                                                     on_update=[]),
                        ))
                    i.sync_info = mybir.SyncInfo(
                        on_wait=[waits[-1]],
                        on_update=list(si.on_update or []))
                    changed = True
                new.append(i)
            if changed:
                b.instructions = new


def _get_nc():
    if "nc" not in _CACHE:
        _CACHE["nc"] = _build()
    return _CACHE["nc"]


def _gather_meta(ts):
    """Per-core gather indices + in-chunk offsets from core-local targets.

    idx16: [P, NG*(GROWS//16)] int16, region g's 2048 chunk ids wrapped as
           id(i) at [i % 16, g*128 + i//16], replicated across the eight
           16-partition groups.
    off:   [P, NT] f32, off[p, j] = (r*C + t_r) % CHUNK for r = j*128+p.
    """
    r = np.arange(BL, dtype=np.int64)
    flat = r * C + ts
    off = (flat % CHUNK).astype(np.float32)
    off = off.reshape(NT, P).T            # [P, NT]

    idx_all = np.empty((P, NG * (GROWS // 16)), dtype=np.int16)
    for g in range(NG):
        fl = flat[g * GROWS:(g + 1) * GROWS] - g * GROWS * C
        cid = (fl // CHUNK).astype(np.int16)          # [2048]
        wrapped = cid.reshape(GROWS // 16, 16).T      # [16, 128]
        block = np.tile(wrapped, (P // 16, 1))        # [128, 128]
        idx_all[:, g * (GROWS // 16):(g + 1) * (GROWS // 16)] = block
    return idx_all, np.ascontiguousarray(off)


def _make_in_maps(input, target):
    x = np.ascontiguousarray(np.asarray(input, dtype=np.float32))
    t = np.asarray(target).astype(np.int64)
    in_maps = []
    for i in range(N_CORES):
        xs = x[i * BL:(i + 1) * BL]
        ts = t[i * BL:(i + 1) * BL]
        idx_all, off = _gather_meta(ts)
        in_maps.append({"x": xs, "idx": idx_all, "off": off})
    return in_maps


def _run(input, target, trace=False):
    from concourse.bass_utils import run_bass_kernel_spmd

    nc = _get_nc()
    in_maps = _make_in_maps(input, target)
    res = run_bass_kernel_spmd(nc, in_maps, list(range(N_CORES)), trace=trace)
    total = np.float64(0.0)
    for r in res.results:
        total += np.float64(r["out"].sum(dtype=np.float64))
    loss = np.float32(total / B)
    return loss, res


def kernel(input, target):
    loss, _ = _run(input, target)
    return loss

